# revision 1
# baseline (speedup 1.0000x reference)
"""BiGN (2-relation LightGCN-style GNN) on 8 Trainium2 NeuronCores.

Strategy (dst-sharded, SPMD):
- Node space padded to 8 x 18816 rows; core k owns rows [k*18816, (k+1)*18816).
- Edges of both relations are routed to the dst-owner core, grouped by
  (8-block super-tile, src 32K chunk, 128-row dst block), padded to x128.
- Per layer: dma_gather rows of the replicated table (HBM) by src index,
  scale by edge value (DVE, bf16), build one-hot dst masks via iota/is_equal
  (DVE, bf16), segment-sum via PE matmul into PSUM, then the dual-relation
  attention combine (DVE/ACT), all per super-tile.
- New embeddings are AllGather'd across the 8 cores into the next layer's
  replicated table. Final layer mean -> per-core light_out slice; host
  assembles and does the tiny batch dot.
"""

import os
import numpy as np
import ml_dtypes

# ---------------------------------------------------------------- constants
NCORES = 8
N_USER = 100000
N_ITEM = 50000
N = N_USER + N_ITEM
D = 64
LAYERS = 3
NODES_PER_CORE_REAL = N // NCORES        # 18750
BLOCKS_PER_CORE = 147
NPC = BLOCKS_PER_CORE * 128              # 18816
NPAD = NCORES * NPC                      # 150528
CHUNK = 32768
NCHUNKS = (NPAD + CHUNK - 1) // CHUNK    # 5
STILE_BLOCKS = 8
NSTILES = (BLOCKS_PER_CORE + STILE_BLOCKS - 1) // STILE_BLOCKS  # 19
PAD_DPOS = 200.0
BF16 = ml_dtypes.bfloat16

_CACHE = {}


# ---------------------------------------------------------------- host prep
def _remap(n):
    return (n // NODES_PER_CORE_REAL) * NPC + (n % NODES_PER_CORE_REAL)


def _preprocess(graph_src, graph_dst, graph_val, sim_src, sim_dst, sim_val):
    sets = []
    for (s, d, v) in ((graph_src, graph_dst, graph_val),
                      (sim_src, sim_dst, sim_val)):
        sp = _remap(s.astype(np.int64))
        dp = _remap(d.astype(np.int64))
        sets.append(dict(core=dp // NPC, blk=(dp % NPC) // 128,
                         dpos=dp % 128, chunk=sp // CHUNK,
                         lsrc=sp % CHUNK, val=v))

    counts = np.zeros((NCORES, 2, BLOCKS_PER_CORE, NCHUNKS), np.int64)
    for si, e in enumerate(sets):
        np.add.at(counts, (e["core"], si, e["blk"], e["chunk"]), 1)
    n_mm = np.maximum(1, -(-counts.max(axis=0) // 128))  # [2, BLK, NCH]

    streams = []
    meta = None
    for c in range(NCORES):
        gidx_parts, val_parts, dpos_parts = [], [], []
        mmeta = []
        sel = {}
        for si, e in enumerate(sets):
            m = e["core"] == c
            key = e["blk"][m].astype(np.int64) * NCHUNKS + e["chunk"][m]
            order = np.argsort(key, kind="stable")
            idx_sorted = np.nonzero(m)[0][order]
            key_sorted = key[order]
            starts = np.searchsorted(
                key_sorted, np.arange(BLOCKS_PER_CORE * NCHUNKS))
            ends = np.searchsorted(
                key_sorted, np.arange(BLOCKS_PER_CORE * NCHUNKS) + 1)
            sel[si] = (idx_sorted, starts, ends)

        for st in range(NSTILES):
            blocks = range(st * STILE_BLOCKS,
                           min((st + 1) * STILE_BLOCKS, BLOCKS_PER_CORE))
            for si, e in enumerate(sets):
                idx_sorted, starts, ends = sel[si]
                for ch in range(NCHUNKS):
                    gi_l, va_l, dp_l = [], [], []
                    for b in blocks:
                        k = b * NCHUNKS + ch
                        eidx = idx_sorted[starts[k]:ends[k]]
                        L = n_mm[si, b, ch] * 128
                        gi = np.zeros(L, np.int16)
                        va = np.zeros(L, np.float32)
                        dpz = np.full(L, PAD_DPOS, np.float32)
                        gi[:len(eidx)] = e["lsrc"][eidx]
                        va[:len(eidx)] = e["val"][eidx]
                        dpz[:len(eidx)] = e["dpos"][eidx]
                        gi_l.append(gi); va_l.append(va); dp_l.append(dpz)
                    gi = np.concatenate(gi_l)
                    va = np.concatenate(va_l)
                    dpz = np.concatenate(dp_l)
                    if c == 0:
                        mmeta.append((st, si, ch, len(gi)))
                    gw = np.ascontiguousarray(gi.reshape(-1, 16).T)
                    gidx_parts.append(np.tile(gw, (8, 1)))
                    val_parts.append(np.ascontiguousarray(
                        va.reshape(-1, 128).T))
                    dpos_parts.append(np.ascontiguousarray(
                        dpz.reshape(-1, 128).T))
        streams.append(dict(
            gidx=np.concatenate(gidx_parts, axis=1),
            val=np.concatenate(val_parts, axis=1),
            dpos=np.concatenate(dpos_parts, axis=1).astype(BF16),
        ))
        if c == 0:
            meta = mmeta
    return n_mm, meta, streams


# ---------------------------------------------------------------- device
def _build_module(n_mm, meta, tot16, tot128):
    import concourse.bacc as bacc
    import concourse.mybir as mybir
    import concourse.tile as tile
    from concourse.library_config import mlp

    f32 = mybir.dt.float32
    bf16 = mybir.dt.bfloat16

    nc = bacc.Bacc("TRN2", target_bir_lowering=False, debug=False,
                   num_devices=NCORES, num_swdge_queues=4)
    emb_slice = nc.dram_tensor("emb_slice", [NPC, D], f32,
                               kind="ExternalInput")
    gidx = nc.dram_tensor("gidx", [128, tot16], mybir.dt.int16,
                          kind="ExternalInput")
    val_in = nc.dram_tensor("val", [128, tot128], f32, kind="ExternalInput")
    dpos_in = nc.dram_tensor("dpos", [128, tot128], bf16,
                             kind="ExternalInput")
    light_out = nc.dram_tensor("light_out", [NPC, D], f32,
                               kind="ExternalOutput")

    # offsets per call in the concatenated streams
    offs = []
    o16 = o128 = 0
    for (st, si, ch, L) in meta:
        offs.append((o16, o128))
        o16 += L // 16
        o128 += L // 128
    assert o16 == tot16 and o128 == tot128
    call_of = {}
    for i, (st, si, ch, L) in enumerate(meta):
        call_of[(st, si, ch)] = (L,) + offs[i]

    with tile.TileContext(nc) as tc:
        nc.gpsimd.load_library(mlp)
        with (
            tc.tile_pool(name="persist", bufs=1) as pers,
            tc.tile_pool(name="gath", bufs=4) as gpool,
            tc.tile_pool(name="small", bufs=8) as spool,
            tc.tile_pool(name="att", bufs=2) as apool,
            tc.tile_pool(name="psum", bufs=4, space="PSUM") as ppool,
            tc.tile_pool(name="dram", bufs=1, space="DRAM") as dram,
            tc.tile_pool(name="dstage", bufs=2, space="DRAM") as dstage,
        ):
            iota_t = pers.tile([128, 128], bf16)
            nc.gpsimd.iota(iota_t[:], [[1, 128]], channel_multiplier=0,
                           allow_small_or_imprecise_dtypes=True)
            light = pers.tile([128, BLOCKS_PER_CORE, D], f32)
            emb_own = pers.tile([128, BLOCKS_PER_CORE, D], f32)
            # emb_own[p, b, :] = emb rows (b*128+p) of this core's slice
            nc.sync.dma_start(
                emb_own[:],
                emb_slice[:].rearrange("(b p) d -> p b d", p=128))

            tables = [dram.tile([NPAD, D], f32, addr_space="Shared",
                                name=f"table{i}") for i in range(LAYERS)]
            stage0 = dstage.tile([NPC, D], f32, tag="stage")
            nc.sync.dma_start(stage0[:], emb_slice[:])
            nc.gpsimd.collective_compute(
                "AllGather", mybir.AluOpType.bypass,
                ins=[stage0.opt()], outs=[tables[0].opt()],
                replica_groups=[list(range(NCORES))])

            gather_q = 0  # round-robin SWDGE queues -> 4 Q7 core pairs
            for layer in range(LAYERS):
                src_table = tables[layer]
                stg = None
                if layer < LAYERS - 1:
                    stg = dstage.tile([NPC, D], f32, tag="stage",
                                      name=f"stage{layer}")
                for st in range(NSTILES):
                    blocks = list(range(
                        st * STILE_BLOCKS,
                        min((st + 1) * STILE_BLOCKS, BLOCKS_PER_CORE)))
                    nblk = len(blocks)
                    ps = {}
                    for si in range(2):
                        ps[si] = ppool.tile([128, STILE_BLOCKS, D], f32,
                                            tag=f"ps{si}", name=f"ps{si}")
                        # zero data; all MMs run start=False. has_written
                        # per element: set -> add onto 0, clear -> overwrite
                        # with own value. Correct either way, order-free.
                        # (start=True would clear has_written for the WHOLE
                        # bank, wiping sibling blocks' accumulation state.)
                        nc.vector.memset(ps[si][:], 0.0)
                        for ch in range(NCHUNKS):
                            L, co16, co128 = call_of[(st, si, ch)]
                            M = L // 128
                            it = spool.tile([128, L // 16], mybir.dt.int16,
                                            tag="gidx")
                            nc.sync.dma_start(
                                it[:], gidx[:, co16:co16 + L // 16])
                            gt = gpool.tile([128, M, D], f32, tag="gath")
                            cbase = ch * CHUNK
                            cend = min(cbase + CHUNK, NPAD)
                            nc.gpsimd.dma_gather(
                                gt[:], src_table[cbase:cend, :], it[:],
                                L, L, D, single_packet=False,
                                queue_num=gather_q)
                            gather_q = (gather_q + 1) % 4
                            vt = spool.tile([128, M], f32, tag="val")
                            nc.sync.dma_start(
                                vt[:], val_in[:, co128:co128 + M])
                            dt = spool.tile([128, M], bf16, tag="dpos")
                            nc.sync.dma_start(
                                dt[:], dpos_in[:, co128:co128 + M])
                            ms = gpool.tile([128, M, D], bf16, tag="msgs")
                            nc.vector.tensor_tensor(
                                ms[:], gt[:],
                                vt[:].unsqueeze(2).to_broadcast([128, M, D]),
                                mybir.AluOpType.mult)
                            oh = gpool.tile([128, M, 128], bf16, tag="oh")
                            nc.vector.tensor_tensor(
                                oh[:],
                                dt[:].unsqueeze(2).to_broadcast([128, M, 128]),
                                iota_t[:].unsqueeze(1).to_broadcast(
                                    [128, M, 128]),
                                mybir.AluOpType.is_equal)
                            m = 0
                            for bl, b in enumerate(blocks):
                                for k in range(n_mm[si, b, ch]):
                                    nc.tensor.matmul(
                                        ps[si][:, bl, :],
                                        oh[:, m, :], ms[:, m, :],
                                        start=False, stop=False,
                                        skip_group_check=True)
                                    m += 1
                            assert m == M

                    # ---- attention / combine for this super-tile
                    sl = (slice(None), slice(0, nblk), slice(None))
                    eo = emb_own[:, blocks[0]:blocks[0] + nblk, :]
                    e1 = apool.tile([128, nblk, D], f32, tag="e1")
                    nc.vector.tensor_scalar_add(e1[:], eo, 1.0)
                    att = {}
                    for si in range(2):
                        tprod = apool.tile([128, nblk, D], f32, tag=f"tp{si}")
                        nc.vector.tensor_tensor(
                            tprod[:], ps[si][sl], e1[:],
                            mybir.AluOpType.mult)
                        red = apool.tile([128, nblk], f32, tag=f"red{si}")
                        nc.vector.tensor_reduce(
                            red[:], tprod[:], mybir.AxisListType.X,
                            mybir.AluOpType.add)
                        a = apool.tile([128, nblk], f32, tag=f"att{si}")
                        nc.scalar.activation(
                            a[:], red[:], mybir.ActivationFunctionType.Exp,
                            scale=1.0 / D)
                        att[si] = a
                    den = apool.tile([128, nblk], f32, tag="den")
                    nc.vector.tensor_add(den[:], att[0][:], att[1][:])
                    rec = apool.tile([128, nblk], f32, tag="rec")
                    nc.vector.reciprocal(rec[:], den[:])
                    w0 = apool.tile([128, nblk], f32, tag="w0")
                    nc.vector.tensor_mul(w0[:], att[0][:], rec[:])
                    w1 = apool.tile([128, nblk], f32, tag="w1")
                    nc.vector.tensor_mul(w1[:], att[1][:], rec[:])
                    t0 = apool.tile([128, nblk, D], f32, tag="t0")
                    nc.vector.tensor_tensor(
                        t0[:], ps[0][sl],
                        w0[:].unsqueeze(2).to_broadcast([128, nblk, D]),
                        mybir.AluOpType.mult)
                    new = apool.tile([128, nblk, D], f32, tag="new")
                    nc.vector.tensor_tensor(
                        new[:], ps[1][sl],
                        w1[:].unsqueeze(2).to_broadcast([128, nblk, D]),
                        mybir.AluOpType.mult)
                    nc.vector.tensor_add(new[:], new[:], t0[:])

                    lsl = light[:, blocks[0]:blocks[0] + nblk, :]
                    if layer == 0:
                        # light = emb0 + new
                        nc.vector.tensor_add(lsl, eo, new[:])
                    else:
                        nc.vector.tensor_add(lsl, lsl, new[:])
                    if layer == LAYERS - 1:
                        fin = apool.tile([128, nblk, D], f32, tag="fin")
                        nc.vector.tensor_scalar_mul(
                            fin[:], lsl, 1.0 / (LAYERS + 1))
                        nc.sync.dma_start(
                            light_out[:].rearrange(
                                "(b p) d -> p b d", p=128)[
                                :, blocks[0]:blocks[0] + nblk, :],
                            fin[:])

                    if layer < LAYERS - 1:
                        # update own rows + stage for all-gather
                        nc.vector.tensor_copy(eo, new[:])
                        nc.sync.dma_start(
                            stg[:].rearrange("(b p) d -> p b d", p=128)[
                                :, blocks[0]:blocks[0] + nblk, :],
                            new[:])
                        if st == NSTILES - 1:
                            nc.gpsimd.collective_compute(
                                "AllGather", mybir.AluOpType.bypass,
                                ins=[stg.opt()],
                                outs=[tables[layer + 1].opt()],
                                replica_groups=[list(range(NCORES))])
    nc.compile()
    return nc


# ---------------------------------------------------------------- entry
def _get_compiled(inputs):
    key = "module"
    if key in _CACHE:
        return _CACHE[key]
    n_mm, meta, streams = _preprocess(
        np.asarray(inputs["graph_src"]), np.asarray(inputs["graph_dst"]),
        np.asarray(inputs["graph_val"]),
        np.asarray(inputs["sim_src"]), np.asarray(inputs["sim_dst"]),
        np.asarray(inputs["sim_val"]))
    tot16 = sum(L // 16 for (_, _, _, L) in meta)
    tot128 = sum(L // 128 for (_, _, _, L) in meta)
    nc = _build_module(n_mm, meta, tot16, tot128)
    _CACHE[key] = (nc, n_mm, meta, streams)
    return _CACHE[key]


def kernel(user_emb, item_emb, graph_src, graph_dst, graph_val,
           sim_src, sim_dst, sim_val, users, items):
    from concourse.bass_utils import run_bass_kernel_spmd
    import concourse.bass_utils as _bu
    trace = bool(int(os.environ.get("BIGN_TRACE", "0")))
    if trace:
        _bu.upload_artifacts = lambda tmpdir: tmpdir

    inputs = dict(user_emb=user_emb, item_emb=item_emb,
                  graph_src=graph_src, graph_dst=graph_dst,
                  graph_val=graph_val, sim_src=sim_src, sim_dst=sim_dst,
                  sim_val=sim_val, users=users, items=items)
    nc, n_mm, meta, streams = _get_compiled(inputs)

    emb0 = np.concatenate([np.asarray(user_emb, np.float32),
                           np.asarray(item_emb, np.float32)], axis=0)
    in_maps = []
    for c in range(NCORES):
        sl = np.zeros((NPC, D), np.float32)
        sl[:NODES_PER_CORE_REAL] = emb0[c * NODES_PER_CORE_REAL:
                                        (c + 1) * NODES_PER_CORE_REAL]
        in_maps.append(dict(emb_slice=sl, gidx=streams[c]["gidx"],
                            val=streams[c]["val"], dpos=streams[c]["dpos"]))

    res = run_bass_kernel_spmd(nc, in_maps, core_ids=list(range(NCORES)),
                               trace=trace)
    if trace and res.exec_time_ns is not None:
        kernel.last_exec_time_ns = res.exec_time_ns
        kernel.last_trace = res.instructions_and_trace

    light = np.zeros((N, D), np.float32)
    for c in range(NCORES):
        light[c * NODES_PER_CORE_REAL:(c + 1) * NODES_PER_CORE_REAL] = \
            res.results[c]["light_out"][:NODES_PER_CORE_REAL]
    ue = light[:N_USER][np.asarray(users)]
    ie = light[N_USER:][np.asarray(items)]
    return (ue * ie).sum(axis=1).astype(np.float32)



# revision 4
# speedup vs baseline: 1.3574x; 1.3574x over previous
"""BiGN (2-relation LightGCN-style GNN) on 8 Trainium2 NeuronCores.

Strategy (dst-sharded, SPMD):
- Node space padded to 8 x 18816 rows; core k owns rows [k*18816, (k+1)*18816).
  Within each core, nodes are permuted CONE-FIRST: nodes that appear in the
  final batch (users/items) occupy rows [0, CONE_ROWS) so the last layer can
  be computed only for those rows.
- Edges of both relations are routed to the dst-owner core, grouped by
  (8-block super-tile, src 32K chunk, 128-row dst block), padded to x128.
- Per layer: dma_gather rows of the replicated table (HBM) by src index,
  scale by edge value (DVE, bf16), build one-hot dst masks via iota/is_equal
  (DVE, bf16), segment-sum via PE matmul into PSUM (emission interleaved
  across (relation, block) to avoid PSUM RAW chains), then the dual-relation
  attention combine (DVE/ACT), all per super-tile.
- Layers 0-1 run over all 147 blocks and AllGather the new embeddings into
  the next layer's replicated table. Layer 2 (the last) runs only over the
  CONE blocks; light_out covers just those rows. Host assembles and does the
  tiny batch dot.
"""

import os
import numpy as np
import ml_dtypes

# ---------------------------------------------------------------- constants
NCORES = 8
N_USER = 100000
N_ITEM = 50000
N = N_USER + N_ITEM
D = 64
LAYERS = 3
NODES_PER_CORE_REAL = N // NCORES        # 18750
BLOCKS_PER_CORE = 147
NPC = BLOCKS_PER_CORE * 128              # 18816
NPAD = NCORES * NPC                      # 150528
CHUNK = 32768
NCHUNKS = (NPAD + CHUNK - 1) // CHUNK    # 5
STILE_BLOCKS = 8
NSTILES = (BLOCKS_PER_CORE + STILE_BLOCKS - 1) // STILE_BLOCKS  # 19
PAD_DPOS = 200.0
BF16 = ml_dtypes.bfloat16

_CACHE = {}


# ---------------------------------------------------------------- host prep
def _cone_permutations(users, items):
    """Per-core cone-first node permutation.

    Returns (row_of [N]->padded global row, cone_rows_per_core list,
    cone_blocks) where cone nodes of core c occupy rows [0, n_cone_c).
    """
    cone = np.zeros(N, bool)
    cone[np.unique(np.asarray(users))] = True
    cone[N_USER + np.unique(np.asarray(items))] = True
    g = np.arange(N, dtype=np.int64)
    core = g // NODES_PER_CORE_REAL
    row_of = np.empty(N, np.int64)
    n_cone = []
    for c in range(NCORES):
        m = core == c
        gc = g[m]
        is_cone = cone[gc]
        order = np.concatenate([gc[is_cone], gc[~is_cone]])
        row_of[order] = c * NPC + np.arange(len(gc))
        n_cone.append(int(is_cone.sum()))
    cone_blocks = (max(n_cone) + 127) // 128
    return row_of, n_cone, cone_blocks


def _build_streams(sets, nblocks, blk_of, dpos_of, keep_mask):
    """Group edges into (stile, rel, chunk, block) cells padded to x128.

    sets: per relation dict with core/blk/dpos/chunk/lsrc/val (already
    row-mapped).  Returns (n_mm [2, nblocks, NCHUNKS], meta, per-core parts).
    meta rows: (st, si, ch, L).
    """
    nst = (nblocks + STILE_BLOCKS - 1) // STILE_BLOCKS
    counts = np.zeros((NCORES, 2, nblocks, NCHUNKS), np.int64)
    for si, e in enumerate(sets):
        m = keep_mask[si]
        np.add.at(counts, (e["core"][m], si, blk_of[si][m], e["chunk"][m]), 1)
    n_mm = np.maximum(1, -(-counts.max(axis=0) // 128))  # [2, nblocks, NCH]

    percore = []
    meta = None
    for c in range(NCORES):
        gidx_parts, val_parts, dpos_parts = [], [], []
        mmeta = []
        sel = {}
        for si, e in enumerate(sets):
            m = (e["core"] == c) & keep_mask[si]
            key = blk_of[si][m].astype(np.int64) * NCHUNKS + e["chunk"][m]
            order = np.argsort(key, kind="stable")
            idx_sorted = np.nonzero(m)[0][order]
            key_sorted = key[order]
            starts = np.searchsorted(key_sorted, np.arange(nblocks * NCHUNKS))
            ends = np.searchsorted(key_sorted,
                                   np.arange(nblocks * NCHUNKS) + 1)
            sel[si] = (idx_sorted, starts, ends)

        for st in range(nst):
            blocks = range(st * STILE_BLOCKS,
                           min((st + 1) * STILE_BLOCKS, nblocks))
            for si, e in enumerate(sets):
                idx_sorted, starts, ends = sel[si]
                for ch in range(NCHUNKS):
                    gi_l, va_l, dp_l = [], [], []
                    for b in blocks:
                        k = b * NCHUNKS + ch
                        eidx = idx_sorted[starts[k]:ends[k]]
                        L = n_mm[si, b, ch] * 128
                        gi = np.zeros(L, np.int16)
                        va = np.zeros(L, np.float32)
                        dpz = np.full(L, PAD_DPOS, np.float32)
                        gi[:len(eidx)] = e["lsrc"][eidx]
                        va[:len(eidx)] = e["val"][eidx]
                        dpz[:len(eidx)] = dpos_of[si][eidx]
                        gi_l.append(gi); va_l.append(va); dp_l.append(dpz)
                    gi = np.concatenate(gi_l)
                    va = np.concatenate(va_l)
                    dpz = np.concatenate(dp_l)
                    if c == 0:
                        mmeta.append((st, si, ch, len(gi)))
                    gw = np.ascontiguousarray(gi.reshape(-1, 16).T)
                    gidx_parts.append(np.tile(gw, (8, 1)))
                    val_parts.append(np.ascontiguousarray(
                        va.reshape(-1, 128).T))
                    dpos_parts.append(np.ascontiguousarray(
                        dpz.reshape(-1, 128).T))
        percore.append((gidx_parts, val_parts, dpos_parts))
        if c == 0:
            meta = mmeta
    return n_mm, meta, percore


def _preprocess(graph_src, graph_dst, graph_val, sim_src, sim_dst, sim_val,
                users, items):
    row_of, n_cone, cone_blocks = _cone_permutations(users, items)
    cone_rows = cone_blocks * 128

    sets = []
    for (s, d, v) in ((graph_src, graph_dst, graph_val),
                      (sim_src, sim_dst, sim_val)):
        sp = row_of[s.astype(np.int64)]
        dp = row_of[d.astype(np.int64)]
        sets.append(dict(core=dp // NPC, rowl=dp % NPC, chunk=sp // CHUNK,
                         lsrc=(sp % CHUNK).astype(np.int16), val=v))

    blk_of = [e["rowl"] // 128 for e in sets]
    dpos_of = [(e["rowl"] % 128).astype(np.float32) for e in sets]
    all_mask = [np.ones(len(e["val"]), bool) for e in sets]
    cone_mask = [e["rowl"] < cone_rows for e in sets]

    n_mm, meta, percore = _build_streams(
        sets, BLOCKS_PER_CORE, blk_of, dpos_of, all_mask)
    n_mm3, meta3, percore3 = _build_streams(
        sets, cone_blocks, blk_of, dpos_of, cone_mask)

    streams = []
    for c in range(NCORES):
        gp, vp, dp_ = percore[c]
        gp3, vp3, dp3 = percore3[c]
        streams.append(dict(
            gidx=np.concatenate(gp + gp3, axis=1),
            val=np.concatenate(vp + vp3, axis=1),
            dpos=np.concatenate(dp_ + dp3, axis=1).astype(BF16),
        ))
    return dict(n_mm=n_mm, meta=meta, n_mm3=n_mm3, meta3=meta3,
                streams=streams, row_of=row_of, n_cone=n_cone,
                cone_blocks=cone_blocks)


# ---------------------------------------------------------------- device
def _build_module(pp):
    import concourse.bacc as bacc
    import concourse.mybir as mybir
    import concourse.tile as tile
    from concourse.library_config import mlp

    f32 = mybir.dt.float32
    bf16 = mybir.dt.bfloat16

    n_mm, meta = pp["n_mm"], pp["meta"]
    n_mm3, meta3 = pp["n_mm3"], pp["meta3"]
    cone_blocks = pp["cone_blocks"]
    cone_rows = cone_blocks * 128
    nst3 = (cone_blocks + STILE_BLOCKS - 1) // STILE_BLOCKS

    # offsets per call in the concatenated streams (full meta then meta3)
    call_of = {}
    o16 = o128 = 0
    for tag, mm in (("full", meta), ("cone", meta3)):
        for (st, si, ch, L) in mm:
            call_of[(tag, st, si, ch)] = (L, o16, o128)
            o16 += L // 16
            o128 += L // 128
    tot16, tot128 = o16, o128

    nc = bacc.Bacc("TRN2", target_bir_lowering=False, debug=False,
                   num_devices=NCORES, num_swdge_queues=4)
    emb_slice = nc.dram_tensor("emb_slice", [NPC, D], f32,
                               kind="ExternalInput")
    gidx = nc.dram_tensor("gidx", [128, tot16], mybir.dt.int16,
                          kind="ExternalInput")
    val_in = nc.dram_tensor("val", [128, tot128], f32, kind="ExternalInput")
    dpos_in = nc.dram_tensor("dpos", [128, tot128], bf16,
                             kind="ExternalInput")
    light_out = nc.dram_tensor("light_out", [cone_rows, D], f32,
                               kind="ExternalOutput")

    with tile.TileContext(nc) as tc:
        nc.gpsimd.load_library(mlp)
        with (
            tc.tile_pool(name="persist", bufs=1) as pers,
            tc.tile_pool(name="gath", bufs=5) as gpool,
            tc.tile_pool(name="ohms", bufs=3) as opool,
            tc.tile_pool(name="small", bufs=10) as spool,
            tc.tile_pool(name="att", bufs=2) as apool,
            tc.tile_pool(name="psum", bufs=4, space="PSUM") as ppool,
            tc.tile_pool(name="dram", bufs=1, space="DRAM") as dram,
            tc.tile_pool(name="dstage", bufs=2, space="DRAM") as dstage,
        ):
            iota_t = pers.tile([128, 128], bf16)
            nc.gpsimd.iota(iota_t[:], [[1, 128]], channel_multiplier=0,
                           allow_small_or_imprecise_dtypes=True)
            light = pers.tile([128, BLOCKS_PER_CORE, D], f32)
            emb_own = pers.tile([128, BLOCKS_PER_CORE, D], f32)
            # emb_own[p, b, :] = emb rows (b*128+p) of this core's slice
            nc.sync.dma_start(
                emb_own[:],
                emb_slice[:].rearrange("(b p) d -> p b d", p=128))

            tables = [dram.tile([NPAD, D], f32, addr_space="Shared",
                                name=f"table{i}") for i in range(LAYERS)]
            stage0 = dstage.tile([NPC, D], f32, tag="stage")
            nc.sync.dma_start(stage0[:], emb_slice[:])
            nc.gpsimd.collective_compute(
                "AllGather", mybir.AluOpType.bypass,
                ins=[stage0.opt()], outs=[tables[0].opt()],
                replica_groups=[list(range(NCORES))])

            gather_q = 0  # round-robin SWDGE queues

            def do_stile(layer, tag, st, blocks, mm_arr):
                nonlocal gather_q
                nblk = len(blocks)
                src_table = tables[layer]
                ps = {}
                for si in range(2):
                    ps[si] = ppool.tile([128, STILE_BLOCKS, D], f32,
                                        tag=f"ps{si}", name=f"ps{si}")
                    # zero data; all MMs run start=False. has_written
                    # per element: set -> add onto 0, clear -> overwrite
                    # with own value. Correct either way, order-free.
                    # (start=True would clear has_written for the WHOLE
                    # bank, wiping sibling blocks' accumulation state.)
                    nc.vector.memset(ps[si][:], 0.0)
                    for ch in range(NCHUNKS):
                        L, co16, co128 = call_of[(tag, st, si, ch)]
                        M = L // 128
                        it = spool.tile([128, L // 16], mybir.dt.int16,
                                        tag="gidx")
                        nc.sync.dma_start(
                            it[:], gidx[:, co16:co16 + L // 16])
                        gt = gpool.tile([128, M, D], f32, tag="gath")
                        cbase = ch * CHUNK
                        cend = min(cbase + CHUNK, NPAD)
                        nc.gpsimd.dma_gather(
                            gt[:], src_table[cbase:cend, :], it[:],
                            L, L, D, single_packet=False,
                            queue_num=gather_q)
                        gather_q = (gather_q + 1) % 4
                        vt = spool.tile([128, M], f32, tag="val")
                        nc.sync.dma_start(
                            vt[:], val_in[:, co128:co128 + M])
                        dt = spool.tile([128, M], bf16, tag="dpos")
                        nc.sync.dma_start(
                            dt[:], dpos_in[:, co128:co128 + M])
                        ms = opool.tile([128, M, D], bf16, tag="msgs")
                        nc.vector.tensor_tensor(
                            ms[:], gt[:],
                            vt[:].unsqueeze(2).to_broadcast([128, M, D]),
                            mybir.AluOpType.mult)
                        oh = opool.tile([128, M, 128], bf16, tag="oh")
                        nc.vector.tensor_tensor(
                            oh[:],
                            dt[:].unsqueeze(2).to_broadcast([128, M, 128]),
                            iota_t[:].unsqueeze(1).to_broadcast(
                                [128, M, 128]),
                            mybir.AluOpType.is_equal)
                        # emit matmuls k-outer/block-inner so consecutive PE
                        # ops hit different PSUM tiles (no RAW accum chain)
                        m_base = {}
                        m = 0
                        for b in blocks:
                            m_base[b] = m
                            m += mm_arr[si, b, ch]
                        assert m == M
                        kmax = max(mm_arr[si, b, ch] for b in blocks)
                        for k in range(kmax):
                            for bl, b in enumerate(blocks):
                                if k < mm_arr[si, b, ch]:
                                    mm = m_base[b] + k
                                    nc.tensor.matmul(
                                        ps[si][:, bl, :],
                                        oh[:, mm, :], ms[:, mm, :],
                                        start=False, stop=False,
                                        skip_group_check=True)
                return ps

            def attention_combine(ps, blocks, eo):
                nblk = len(blocks)
                sl = (slice(None), slice(0, nblk), slice(None))
                e1 = apool.tile([128, nblk, D], f32, tag="e1")
                nc.vector.tensor_scalar_add(e1[:], eo, 1.0)
                att = {}
                for si in range(2):
                    tprod = apool.tile([128, nblk, D], f32, tag=f"tp{si}")
                    nc.vector.tensor_tensor(
                        tprod[:], ps[si][sl], e1[:], mybir.AluOpType.mult)
                    red = apool.tile([128, nblk], f32, tag=f"red{si}")
                    nc.vector.tensor_reduce(
                        red[:], tprod[:], mybir.AxisListType.X,
                        mybir.AluOpType.add)
                    a = apool.tile([128, nblk], f32, tag=f"att{si}")
                    nc.scalar.activation(
                        a[:], red[:], mybir.ActivationFunctionType.Exp,
                        scale=1.0 / D)
                    att[si] = a
                den = apool.tile([128, nblk], f32, tag="den")
                nc.vector.tensor_add(den[:], att[0][:], att[1][:])
                rec = apool.tile([128, nblk], f32, tag="rec")
                nc.vector.reciprocal(rec[:], den[:])
                w0 = apool.tile([128, nblk], f32, tag="w0")
                nc.vector.tensor_mul(w0[:], att[0][:], rec[:])
                w1 = apool.tile([128, nblk], f32, tag="w1")
                nc.vector.tensor_mul(w1[:], att[1][:], rec[:])
                t0 = apool.tile([128, nblk, D], f32, tag="t0")
                nc.vector.tensor_tensor(
                    t0[:], ps[0][sl],
                    w0[:].unsqueeze(2).to_broadcast([128, nblk, D]),
                    mybir.AluOpType.mult)
                new = apool.tile([128, nblk, D], f32, tag="new")
                nc.vector.tensor_tensor(
                    new[:], ps[1][sl],
                    w1[:].unsqueeze(2).to_broadcast([128, nblk, D]),
                    mybir.AluOpType.mult)
                nc.vector.tensor_add(new[:], new[:], t0[:])
                return new

            # ---- layers 0..LAYERS-2: full node range
            for layer in range(LAYERS - 1):
                stg = dstage.tile([NPC, D], f32, tag="stage",
                                  name=f"stage{layer}")
                for st in range(NSTILES):
                    blocks = list(range(
                        st * STILE_BLOCKS,
                        min((st + 1) * STILE_BLOCKS, BLOCKS_PER_CORE)))
                    nblk = len(blocks)
                    ps = do_stile(layer, "full", st, blocks, n_mm)
                    eo = emb_own[:, blocks[0]:blocks[0] + nblk, :]
                    new = attention_combine(ps, blocks, eo)
                    lsl = light[:, blocks[0]:blocks[0] + nblk, :]
                    if layer == 0:
                        nc.vector.tensor_add(lsl, eo, new[:])
                    else:
                        nc.vector.tensor_add(lsl, lsl, new[:])
                    # update own rows + stage for all-gather
                    nc.vector.tensor_copy(eo, new[:])
                    nc.sync.dma_start(
                        stg[:].rearrange("(b p) d -> p b d", p=128)[
                            :, blocks[0]:blocks[0] + nblk, :],
                        new[:])
                    if st == NSTILES - 1:
                        nc.gpsimd.collective_compute(
                            "AllGather", mybir.AluOpType.bypass,
                            ins=[stg.opt()],
                            outs=[tables[layer + 1].opt()],
                            replica_groups=[list(range(NCORES))])

            # ---- final layer: cone blocks only
            for st in range(nst3):
                blocks = list(range(
                    st * STILE_BLOCKS,
                    min((st + 1) * STILE_BLOCKS, cone_blocks)))
                nblk = len(blocks)
                ps = do_stile(LAYERS - 1, "cone", st, blocks, n_mm3)
                eo = emb_own[:, blocks[0]:blocks[0] + nblk, :]
                new = attention_combine(ps, blocks, eo)
                lsl = light[:, blocks[0]:blocks[0] + nblk, :]
                nc.vector.tensor_add(new[:], lsl, new[:])
                fin = apool.tile([128, nblk, D], f32, tag="fin")
                nc.vector.tensor_scalar_mul(fin[:], new[:],
                                            1.0 / (LAYERS + 1))
                nc.sync.dma_start(
                    light_out[:].rearrange("(b p) d -> p b d", p=128)[
                        :, blocks[0]:blocks[0] + nblk, :],
                    fin[:])
    nc.compile()
    return nc


# ---------------------------------------------------------------- entry
def _get_compiled(inputs):
    key = "module"
    if key in _CACHE:
        return _CACHE[key]
    pp = _preprocess(
        np.asarray(inputs["graph_src"]), np.asarray(inputs["graph_dst"]),
        np.asarray(inputs["graph_val"]),
        np.asarray(inputs["sim_src"]), np.asarray(inputs["sim_dst"]),
        np.asarray(inputs["sim_val"]),
        np.asarray(inputs["users"]), np.asarray(inputs["items"]))
    nc = _build_module(pp)
    _CACHE[key] = (nc, pp)
    return _CACHE[key]


def kernel(user_emb, item_emb, graph_src, graph_dst, graph_val,
           sim_src, sim_dst, sim_val, users, items):
    from concourse.bass_utils import run_bass_kernel_spmd
    import concourse.bass_utils as _bu
    trace = bool(int(os.environ.get("BIGN_TRACE", "0")))
    if trace:
        _bu.upload_artifacts = lambda tmpdir: tmpdir

    inputs = dict(user_emb=user_emb, item_emb=item_emb,
                  graph_src=graph_src, graph_dst=graph_dst,
                  graph_val=graph_val, sim_src=sim_src, sim_dst=sim_dst,
                  sim_val=sim_val, users=users, items=items)
    nc, pp = _get_compiled(inputs)
    row_of = pp["row_of"]
    cone_rows = pp["cone_blocks"] * 128

    emb0 = np.concatenate([np.asarray(user_emb, np.float32),
                           np.asarray(item_emb, np.float32)], axis=0)
    in_maps = []
    for c in range(NCORES):
        sl = np.zeros((NPC, D), np.float32)
        gs = np.arange(c * NODES_PER_CORE_REAL,
                       (c + 1) * NODES_PER_CORE_REAL)
        sl[row_of[gs] - c * NPC] = emb0[gs]
        in_maps.append(dict(emb_slice=sl, gidx=pp["streams"][c]["gidx"],
                            val=pp["streams"][c]["val"],
                            dpos=pp["streams"][c]["dpos"]))

    res = run_bass_kernel_spmd(nc, in_maps, core_ids=list(range(NCORES)),
                               trace=trace)
    if trace and res.exec_time_ns is not None:
        kernel.last_exec_time_ns = res.exec_time_ns
        kernel.last_trace = res.instructions_and_trace

    users_a = np.asarray(users)
    items_a = np.asarray(items)
    need = np.concatenate([np.unique(users_a),
                           N_USER + np.unique(items_a)])
    light = np.zeros((N, D), np.float32)
    rows = row_of[need]
    cores = rows // NPC
    local = rows % NPC
    assert (local < cone_rows).all()
    for c in range(NCORES):
        m = cores == c
        light[need[m]] = res.results[c]["light_out"][local[m]]
    ue = light[:N_USER][users_a]
    ie = light[N_USER:][items_a]
    return (ue * ie).sum(axis=1).astype(np.float32)
